# revision 1
# baseline (speedup 1.0000x reference)
"""Trainium2 Bass kernel for AngelLoss (center loss + angular loss).

loss = 0.5*sum((feat - centers[y])^2)/B
     + sum_offdiag((c_i.c_j/(|c_i||c_j|) - ct)^2) / (0.5*C*(C-1))

Sharding (8 NeuronCores):
  - batch term: feat/y sharded along batch (8192 rows/core); each core
    dma_gathers its centers rows (bf16 table) in 9 chunks, subtracts on
    DVE, and square-reduces on ScalarE into per-partition partials.
  - angular term: center rows sharded (125 rows/core); each core computes
    its 125x1000 slice of the normalized Gram matrix on TensorE.
  - per-core [1,16] partial sums are combined on the host.
"""

from contextlib import ExitStack

import ml_dtypes
import numpy as np

import concourse.bass as bass
import concourse.tile as tile
from concourse import bacc, mybir
from concourse.bass import ds, ts
from concourse.bass_utils import run_bass_kernel_spmd

N_CORES = 8
B, C, D = 65536, 1000, 512
BS = B // N_CORES  # 8192 rows per core
CHUNK_SIZES = [1024] * 7 + [512, 512]  # small tail chunks shorten the serial tail
CHUNKS = len(CHUNK_SIZES)
MAXSLOTS = max(CHUNK_SIZES) // 128
CS = C // N_CORES  # 125 gram rows per core

# ct = 2*radius(C-1)^2 - 1 from the reference, evaluated in f64, cast f32.
CT = float(np.float32(-0.0010010010010047532))

_F32 = mybir.dt.float32
_BF16 = mybir.dt.bfloat16
_I16 = mybir.dt.int16

_NC_CACHE = {}


def _build_body(ctx, tc, feat, cslice, idx16, identity, cbf, out):
    nc = tc.nc
    AF = mybir.ActivationFunctionType

    const = ctx.enter_context(tc.tile_pool(name="const", bufs=1))
    pnrm = ctx.enter_context(tc.tile_pool(name="nrm", bufs=3))
    pfeat = ctx.enter_context(tc.tile_pool(name="feat", bufs=3))
    pgath = ctx.enter_context(tc.tile_pool(name="gath", bufs=4))
    pscr = ctx.enter_context(tc.tile_pool(name="scr", bufs=2))
    ptp = ctx.enter_context(tc.tile_pool(name="tp", bufs=2, space="PSUM"))
    pgram = ctx.enter_context(tc.tile_pool(name="gram", bufs=2, space="PSUM"))
    pfin = ctx.enter_context(tc.tile_pool(name="fin", bufs=1, space="PSUM"))

    idxt = const.tile([128, BS // 16], _I16)
    nc.sync.dma_start(idxt[:], idx16[:, :])

    # Warm up the SWDGE gather path (library load + Q7 code fetch) while the
    # table loads. Keep ALL other work off gpsimd so the library switch is
    # the very first thing the engine does.
    warm = const.tile([128, 1, D], _BF16)
    nc.gpsimd.dma_gather(warm[:], cbf[:, :], idxt[:, 0:8], 128, 128, D)

    # identity comes in as an input; constants built on DVE (NOT gpsimd,
    # to keep the mlp-library switch at the head of the gpsimd stream)
    ident = const.tile([128, 128], _BF16)
    nc.sync.dma_start(ident[:], identity[:, :])
    ones = const.tile([128, 1], _F32)
    nc.vector.memset(ones[:], 1.0)
    # staging[:, 0:8]: per-chunk center-loss partials; [:, 8:10]: angular
    # halves; [:, 15]: warmup-gather consumer (ignored by the host).
    staging = const.tile([128, 16], _F32)
    nc.vector.memset(staging[:], 0.0)
    ctbias = const.tile([128, 1], _F32)
    nc.vector.memset(ctbias[:], -CT)

    # --- one-shot centers load -> bf16 table store (cbf) ---
    # partition p holds rows p*8 .. p*8+7 (contiguous 16 KiB per partition)
    ct_all = const.tile([125, 8, D], _BF16)
    nc.scalar.dma_start(ct_all[:], cbf.rearrange("(p s) d -> p s d", p=125))
    tslice = const.tile([128, D], _F32)
    nc.sync.dma_start(tslice[:CS, :], cslice[:, :])

    # --- angular term (fills the window while gathers/feat stream) ---
    # normalize the [125, 8, D] resident center rows; per-(p,s) row norms via
    # ScalarE square + fused free-axis accumulate, one slot column at a time
    nsq = const.tile([125, 8], _F32)
    for s in range(8):
        ttr = pnrm.tile([125, D], _F32, tag="ttr")
        nc.scalar.activation(
            ttr[:], ct_all[:, s, :], AF.Square, accum_out=nsq[:, s : s + 1]
        )
    sd = pnrm.tile([125, 8], _F32, tag="sd")
    nc.scalar.activation(sd[:], nsq[:], AF.Sqrt)
    inv = pnrm.tile([125, 8], _F32, tag="inv")
    nc.vector.reciprocal(inv[:], sd[:])
    cn_all = const.tile([125, 8, D], _BF16)
    nc.vector.tensor_tensor(
        out=cn_all[:],
        in0=ct_all[:],
        in1=inv[:, :].to_broadcast([125, 8, D]),
        op=mybir.AluOpType.mult,
    )
    # normalize the slice rows ([125, D], one row per partition)
    sq2 = pnrm.tile([128, D], _F32, tag="sq2")
    nsq2 = pnrm.tile([128, 1], _F32, tag="nsq2")
    nc.scalar.activation(sq2[:CS, :], tslice[:CS, :], AF.Square, accum_out=nsq2[:CS, :])
    sd2 = pnrm.tile([128, 1], _F32, tag="sd2")
    nc.scalar.activation(sd2[:CS, :], nsq2[:CS, :], AF.Sqrt)
    inv2 = pnrm.tile([128, 1], _F32, tag="inv2")
    nc.vector.reciprocal(inv2[:CS, :], sd2[:CS, :])
    cns = const.tile([128, D], _BF16)
    nc.scalar.activation(cns[:CS, :], tslice[:CS, :], AF.Copy, scale=inv2[:CS, :])

    # transposed normalized centers; class (q, s) = row q*8+s in column (q, s)
    _hp = tc.high_priority()
    _hp.__enter__()
    cnT = const.tile([128, 4, 125, 8], _BF16)
    for s in range(8):
        for ki in range(4):
            pt = ptp.tile([128, 128], _BF16, tag="tp")
            nc.tensor.transpose(pt[:, :125], cn_all[:, s, ts(ki, 128)], ident[:125, :125])
            nc.vector.tensor_copy(cnT[:, ki, :, s : s + 1], pt[:, :125])
    cnTs = const.tile([128, 4, CS], _BF16)
    for ki in range(4):
        pt = ptp.tile([128, 128], _BF16, tag="tp")
        nc.tensor.transpose(pt[:, :CS], cns[:CS, ts(ki, 128)], ident[:CS, :CS])
        nc.vector.tensor_copy(cnTs[:, ki, :], pt[:, :CS])

    # gram slice: [125 rows, 1000 classes] in two column halves
    HALVES = [(0, 63), (63, 62)]
    grams = []
    for q0, qn in HALVES:
        pg = pgram.tile([CS, 63 * 8], _F32, tag="gram")
        for ki in range(4):
            nc.tensor.matmul(
                pg[:, : qn * 8],
                cnTs[:, ki, :],
                cnT[:, ki, ds(q0, qn), :],
                start=(ki == 0),
                stop=(ki == 3),
            )
        grams.append((pg, qn))
    _hp.__exit__(None, None, None)

    # --- center loss loop ---
    row0 = 0
    for c, csz in enumerate(CHUNK_SIZES):
        slots = csz // 128
        gt = pgath.tile([128, MAXSLOTS, D], _BF16, tag="gt")
        nc.gpsimd.dma_gather(
            gt[:, :slots, :],
            cbf[:, :],
            idxt[:, ds(row0 // 16, csz // 16)],
            csz,
            csz,
            D,
        )
        ft = pfeat.tile([128, MAXSLOTS, D], _F32, tag="ft")
        # partition p holds rows [row0 + p*slots, ...+slots): contiguous
        nc.sync.dma_start(
            ft[:, :slots, :],
            feat[ds(row0, csz), :].rearrange("(p s) d -> p s d", p=128),
        )
        st = pscr.tile([128, MAXSLOTS, D], _F32, tag="st")
        nc.vector.tensor_tensor(
            out=st[:, :slots, :],
            in0=ft[:, :slots, :],
            in1=gt[:, :slots, :],
            op=mybir.AluOpType.subtract,
        )
        nc.scalar.activation(
            ft[:, :slots, :],
            st[:, :slots, :],
            AF.Square,
            accum_out=staging[:, c : c + 1],
        )
        row0 += csz

    # --- angular squares + final partition collapse ---
    for h, (pg, qn) in enumerate(grams):
        gs = pnrm.tile([CS, 63 * 8], _F32, tag="gscr")
        nc.scalar.activation(
            gs[:, : qn * 8],
            pg[:, : qn * 8],
            AF.Square,
            bias=ctbias[:CS, :],
            accum_out=staging[:CS, 10 + h : 11 + h],
        )
    pf = pfin.tile([1, 16], _F32, tag="fin")
    nc.tensor.matmul(pf[:], ones[:], staging[:], start=True, stop=True)
    osb = const.tile([1, 16], _F32)
    nc.vector.tensor_copy(osb[:], pf[:])
    nc.vector.tensor_copy(osb[0:1, 15:16], warm[0:1, 0, 0:1])
    nc.sync.dma_start(out[:, :], osb[:, :])


def build():
    if "nc" in _NC_CACHE:
        return _NC_CACHE["nc"]
    nc = bacc.Bacc(
        "TRN2",
        target_bir_lowering=False,
        debug=False,
        enable_asserts=False,
        num_devices=N_CORES,
    )
    feat = nc.dram_tensor("feat", [BS, D], _F32, kind="ExternalInput").ap()
    cslice = nc.dram_tensor("cslice", [CS, D], _F32, kind="ExternalInput").ap()
    idx16 = nc.dram_tensor("idx16", [128, BS // 16], _I16, kind="ExternalInput").ap()
    identity = nc.dram_tensor("identity", [128, 128], _BF16, kind="ExternalInput").ap()
    cbf = nc.dram_tensor("ctab", [C, D], _BF16, kind="ExternalInput").ap()
    out = nc.dram_tensor("out", [1, 16], _F32, kind="ExternalOutput").ap()
    with tile.TileContext(nc) as tc, ExitStack() as ctx:
        _build_body(ctx, tc, feat, cslice, idx16, identity, cbf, out)
    nc.compile()
    _NC_CACHE["nc"] = nc
    return nc


def make_in_maps(y, feat, centers):
    feat = np.ascontiguousarray(feat, dtype=np.float32)
    centers = np.ascontiguousarray(centers, dtype=np.float32)
    y = np.asarray(y)
    ctab = centers.astype(ml_dtypes.bfloat16)
    in_maps = []
    for i in range(N_CORES):
        ys = y[i * BS : (i + 1) * BS].astype(np.int16)
        # gather position j in a chunk pairs with feat row row0 + (j%128)*slots + j//128
        parts = []
        row0 = 0
        for csz in CHUNK_SIZES:
            slots = csz // 128
            j = np.arange(csz)
            parts.append(ys[row0 + (j % 128) * slots + j // 128])
            row0 += csz
        yp = np.concatenate(parts)
        # [16, BS/16] stripes (position j at [j%16, j//16]), replicated into all
        # eight 16-partition groups (each SWDGE Q7 core reads its own stripe).
        idx = np.tile(yp.reshape(BS // 16, 16).T, (8, 1))
        in_maps.append(
            {
                "feat": np.ascontiguousarray(feat[i * BS : (i + 1) * BS]),
                "cslice": np.ascontiguousarray(centers[i * CS : (i + 1) * CS]),
                "idx16": idx,
                "identity": np.eye(128, dtype=ml_dtypes.bfloat16),
                "ctab": ctab,
            }
        )
    return in_maps


def combine(outs):
    """outs: list of 8 [1,16] f32 arrays -> scalar loss (np.float32)."""
    cen = 0.0
    ang = 0.0
    for o in outs:
        o = np.asarray(o, dtype=np.float64)
        cen += o[0, 0:9].sum()
        ang += o[0, 10:12].sum()
    ang -= C * (1.0 - CT) ** 2  # remove the diagonal (sim_ii == 1) terms
    loss = 0.5 * cen / B + ang / (0.5 * C * (C - 1))
    return np.float32(loss)


def kernel(y, feat, centers):
    nc = build()
    in_maps = make_in_maps(y, feat, centers)
    res = run_bass_kernel_spmd(nc, in_maps, core_ids=list(range(N_CORES)))
    return combine([res.results[i]["out"] for i in range(N_CORES)])



# revision 3
# speedup vs baseline: 1.4069x; 1.4069x over previous
"""Trainium2 Bass kernel for AngelLoss (center loss + angular loss).

loss = 0.5*sum((feat - centers[y])^2)/B
     + sum_offdiag((c_i.c_j/(|c_i||c_j|) - ct)^2) / (0.5*C*(C-1))

Strategy (8 NeuronCores):
  - The batch sum is order-invariant, so the host shards feat GLOBALLY
    SORTED by class (index prep only; all O(B*D) compute stays on
    device). Each 128-row chunk then spans <= ~4 consecutive classes,
    and centers[y] is expanded on-device by a tiny one-hot matmul on
    TensorE against an 8-row class window -- no SWDGE gather at all.
  - Per chunk: PSUM diff source = onehot^T @ window (TensorE), subtract
    vs feat (DVE), square+accumulate (ScalarE).
  - Angular term: center rows sharded (125 rows/core); the host uploads
    the normalized center table pre-transposed (d-major), so the
    125x1000 Gram slice is 8 direct matmuls -- no on-device transposes.
  - per-core [1,20] partial sums are combined on the host.
"""

from contextlib import ExitStack

import ml_dtypes
import numpy as np

import concourse.bass as bass
import concourse.tile as tile
from concourse import bacc, mybir
from concourse.bass import ds, ts
from concourse.bass_utils import run_bass_kernel_spmd

N_CORES = 8
B, C, D = 65536, 1000, 512
BS = B // N_CORES  # 8192 rows per core
CS = C // N_CORES  # 125 gram rows per core
NW = BS // 128  # 64 one-hot windows per core
WCAP = 8  # max classes spanned by one 128-row sorted window (seed-0 max is 4)
NSC = NW // 4  # 16 superchunks of 4 windows
NSTG = 20  # staging columns: 0..15 center partials, 16..17 angular halves

# ct = 2*radius(C-1)^2 - 1 from the reference, evaluated in f64, cast f32.
CT = float(np.float32(-0.0010010010010047532))

_F32 = mybir.dt.float32
_BF16 = mybir.dt.bfloat16

_NC_CACHE = {}


def _build_body(ctx, tc, feat, oh, wt, ctnT, csT, out):
    nc = tc.nc
    AF = mybir.ActivationFunctionType

    const = ctx.enter_context(tc.tile_pool(name="const", bufs=1))
    pfeat = ctx.enter_context(tc.tile_pool(name="feat", bufs=3))
    pst = ctx.enter_context(tc.tile_pool(name="st", bufs=3))
    pdump = ctx.enter_context(tc.tile_pool(name="dump", bufs=2))
    pang = ctx.enter_context(tc.tile_pool(name="ang", bufs=2))
    pdiff = ctx.enter_context(tc.tile_pool(name="diff", bufs=2, space="PSUM"))
    pgram = ctx.enter_context(tc.tile_pool(name="gram", bufs=2, space="PSUM"))
    pfin = ctx.enter_context(tc.tile_pool(name="fin", bufs=1, space="PSUM"))

    # resident loads (off the feat queue so they overlap the stream)
    oht = const.tile([WCAP, NW, 128], _BF16)
    nc.scalar.dma_start(oht[:], oh[:, :, :])
    wtt = const.tile([WCAP, NW, D], _BF16)
    nc.scalar.dma_start(wtt[:], wt[:, :, :])
    ctn = const.tile([128, 4, C], _BF16)
    nc.scalar.dma_start(ctn[:], ctnT[:, :, :])
    cst = const.tile([128, 4, CS], _BF16)
    nc.scalar.dma_start(cst[:], csT[:, :, :])

    ones = const.tile([128, 1], _F32)
    nc.vector.memset(ones[:], 1.0)
    ctb = const.tile([128, 1], _F32)
    nc.vector.memset(ctb[:], -CT)
    staging = const.tile([128, NSTG], _F32)
    nc.vector.memset(staging[:], 0.0)

    def superchunk(sc):
        ft = pfeat.tile([128, 4, D], _F32, tag="ft")
        nc.sync.dma_start(
            ft[:], feat[ds(sc * 512, 512), :].rearrange("(j b) d -> b j d", b=128)
        )
        st = pst.tile([128, 4, D], _BF16, tag="st")
        for a in range(2):
            pd = pdiff.tile([128, 2, D], _F32, tag="pd")
            for jj in range(2):
                w = sc * 4 + a * 2 + jj
                nc.tensor.matmul(
                    pd[:, jj, :], oht[:, w, :], wtt[:, w, :], start=True, stop=True
                )
            nc.vector.tensor_tensor(
                out=st[:, a * 2 : a * 2 + 2, :],
                in0=ft[:, a * 2 : a * 2 + 2, :],
                in1=pd[:, :, :],
                op=mybir.AluOpType.subtract,
            )
        dmp = pdump.tile([128, 4, D], _BF16, tag="dmp")
        nc.scalar.activation(
            dmp[:], st[:], AF.Square, accum_out=staging[:, sc : sc + 1]
        )

    def angular():
        for h in range(2):
            pg = pgram.tile([CS, 500], _F32, tag="pg")
            for ki in range(4):
                nc.tensor.matmul(
                    pg[:],
                    cst[:, ki, :],
                    ctn[:, ki, ds(500 * h, 500)],
                    start=(ki == 0),
                    stop=(ki == 3),
                )
            gs = pang.tile([CS, 500], _F32, tag="gs")
            nc.scalar.activation(
                gs[:],
                pg[:],
                AF.Square,
                bias=ctb[:CS, :],
                accum_out=staging[:CS, 16 + h : 17 + h],
            )

    superchunk(0)
    superchunk(1)
    angular()
    for sc in range(2, NSC):
        superchunk(sc)

    pf = pfin.tile([1, NSTG], _F32, tag="fin")
    nc.tensor.matmul(pf[:], ones[:], staging[:], start=True, stop=True)
    osb = const.tile([1, NSTG], _F32)
    nc.vector.tensor_copy(osb[:], pf[:])
    nc.sync.dma_start(out[:, :], osb[:, :])


def build():
    if "nc" in _NC_CACHE:
        return _NC_CACHE["nc"]
    nc = bacc.Bacc(
        "TRN2",
        target_bir_lowering=False,
        debug=False,
        enable_asserts=False,
        num_devices=N_CORES,
    )
    feat = nc.dram_tensor("feat", [BS, D], _F32, kind="ExternalInput").ap()
    oh = nc.dram_tensor("oh", [WCAP, NW, 128], _BF16, kind="ExternalInput").ap()
    wt = nc.dram_tensor("wt", [WCAP, NW, D], _BF16, kind="ExternalInput").ap()
    ctnT = nc.dram_tensor("ctnT", [128, 4, C], _BF16, kind="ExternalInput").ap()
    csT = nc.dram_tensor("csT", [128, 4, CS], _BF16, kind="ExternalInput").ap()
    out = nc.dram_tensor("out", [1, NSTG], _F32, kind="ExternalOutput").ap()
    with tile.TileContext(nc) as tc, ExitStack() as ctx:
        _build_body(ctx, tc, feat, oh, wt, ctnT, csT, out)
    nc.compile()
    _NC_CACHE["nc"] = nc
    return nc


def _dmajor(a):
    """[R, 512] -> [128, 4, R]: out[p, k, r] = a[r, 128*k + p]."""
    r = a.shape[0]
    return np.ascontiguousarray(a.T.reshape(4, 128, r).transpose(1, 0, 2))


def make_in_maps(y, feat, centers):
    feat = np.asarray(feat, dtype=np.float32)
    centers = np.asarray(centers, dtype=np.float32)
    y = np.asarray(y)

    order = np.argsort(y, kind="stable")
    ys = np.asarray(y)[order].astype(np.int32)

    norms = np.sqrt((centers.astype(np.float64) ** 2).sum(1)).astype(np.float32)
    ctn = (centers / norms[:, None]).astype(ml_dtypes.bfloat16)
    ctnT = _dmajor(ctn)  # [128, 4, 1000]
    cbf = centers.astype(ml_dtypes.bfloat16)

    in_maps = []
    for i in range(N_CORES):
        sl = slice(i * BS, (i + 1) * BS)
        fs = np.ascontiguousarray(feat[order[sl]])
        lid2 = ys[sl].reshape(NW, 128)
        c0 = lid2[:, 0]  # sorted -> min class of each window
        lid = lid2 - c0[:, None]
        if lid.max() >= WCAP:
            raise ValueError(f"window span {lid.max() + 1} exceeds WCAP={WCAP}")
        rows = np.minimum(c0[None, :] + np.arange(WCAP)[:, None], C - 1)  # [WCAP, NW]
        wt = np.ascontiguousarray(cbf[rows])  # [WCAP, NW, 512]
        oh = (np.arange(WCAP)[:, None, None] == lid[None, :, :]).astype(
            ml_dtypes.bfloat16
        )  # [WCAP, NW, 128]
        csT = _dmajor(np.asarray(ctn[i * CS : (i + 1) * CS]))  # [128, 4, 125]
        in_maps.append(
            {"feat": fs, "oh": oh, "wt": wt, "ctnT": ctnT, "csT": csT}
        )
    return in_maps


def combine(outs):
    """outs: list of 8 [1,NSTG] f32 arrays -> scalar loss (np.float32)."""
    cen = 0.0
    ang = 0.0
    for o in outs:
        o = np.asarray(o, dtype=np.float64)
        cen += o[0, 0:16].sum()
        ang += o[0, 16:18].sum()
    ang -= C * (1.0 - CT) ** 2  # remove the diagonal (sim_ii == 1) terms
    loss = 0.5 * cen / B + ang / (0.5 * C * (C - 1))
    return np.float32(loss)


def kernel(y, feat, centers):
    nc = build()
    in_maps = make_in_maps(y, feat, centers)
    res = run_bass_kernel_spmd(nc, in_maps, core_ids=list(range(N_CORES)))
    return combine([res.results[i]["out"] for i in range(N_CORES)])


# revision 5
# speedup vs baseline: 1.5960x; 1.1344x over previous
"""Trainium2 Bass kernel for AngelLoss (center loss + angular loss).

loss = 0.5*sum((feat - centers[y])^2)/B
     + sum_offdiag((c_i.c_j/(|c_i||c_j|) - ct)^2) / (0.5*C*(C-1))

Strategy (8 NeuronCores):
  - The batch sum is order-invariant, so the host shards feat GLOBALLY
    SORTED by class (index prep only; all O(B*D) compute stays on
    device). Each 128-row chunk then spans <= ~4 consecutive classes,
    and centers[y] is expanded on-device by a tiny one-hot matmul on
    TensorE against an 8-row class window -- no SWDGE gather at all.
    The one-hot and window tables are fp8e4 and the expand matmuls run
    in DoubleRow perf mode (0.5 cycles/row).
  - Per chunk: PSUM diff source = onehot^T @ window (TensorE), subtract
    vs feat (DVE), square+accumulate (ScalarE).
  - Angular term: center rows sharded (125 rows/core); the host uploads
    the normalized center table pre-transposed (d-major), so the
    125x1000 Gram slice is 8 direct matmuls -- no on-device transposes.
  - per-core [1,20] partial sums are combined on the host.
"""

from contextlib import ExitStack

import ml_dtypes
import numpy as np

import concourse.bass as bass
import concourse.tile as tile
from concourse import bacc, mybir
from concourse.bass import ds, ts
from concourse.bass_utils import run_bass_kernel_spmd

N_CORES = 8
B, C, D = 65536, 1000, 512
BS = B // N_CORES  # 8192 rows per core
CS = C // N_CORES  # 125 gram rows per core
NW = BS // 128  # 64 one-hot windows per core
WCAP = 8  # max classes spanned by one 128-row sorted window (seed-0 max is 4)
NSC = NW // 4  # 16 superchunks of 4 windows
NSTG = 20  # staging columns: 0..15 center partials, 16..17 angular halves

# ct = 2*radius(C-1)^2 - 1 from the reference, evaluated in f64, cast f32.
CT = float(np.float32(-0.0010010010010047532))

_F32 = mybir.dt.float32
_BF16 = mybir.dt.bfloat16
_FP8 = mybir.dt.float8e4

_NC_CACHE = {}


def _build_body(ctx, tc, feat, oh, wt, ctnT, csT, out):
    nc = tc.nc
    AF = mybir.ActivationFunctionType
    DR = mybir.MatmulPerfMode.DoubleRow

    const = ctx.enter_context(tc.tile_pool(name="const", bufs=1))
    pfeat = ctx.enter_context(tc.tile_pool(name="feat", bufs=4))
    pst = ctx.enter_context(tc.tile_pool(name="st", bufs=3))
    pdump = ctx.enter_context(tc.tile_pool(name="dump", bufs=2))
    pang = ctx.enter_context(tc.tile_pool(name="ang", bufs=2))
    pdiff = ctx.enter_context(tc.tile_pool(name="diff", bufs=3, space="PSUM"))
    pgram = ctx.enter_context(tc.tile_pool(name="gram", bufs=1, space="PSUM"))
    pfin = ctx.enter_context(tc.tile_pool(name="fin", bufs=1, space="PSUM"))

    # resident loads (off the feat queue so they overlap the stream)
    oht = const.tile([4, NW, 2, 128], _FP8)
    nc.scalar.dma_start(oht[:], oh[:, :, :, :])
    wtt = const.tile([4, NW, 2, D], _FP8)
    nc.scalar.dma_start(wtt[:], wt[:, :, :, :])
    ctn = const.tile([128, 4, C], _BF16)
    nc.scalar.dma_start(ctn[:], ctnT[:, :, :])
    cst = const.tile([128, 4, CS], _BF16)
    nc.scalar.dma_start(cst[:], csT[:, :, :])

    ones = const.tile([128, 1], _F32)
    nc.vector.memset(ones[:], 1.0)
    ctb = const.tile([128, 1], _F32)
    nc.vector.memset(ctb[:], -CT)
    staging = const.tile([128, NSTG], _F32)
    nc.vector.memset(staging[:], 0.0)

    def superchunk(sc):
        ft = pfeat.tile([128, 4, D], _F32, tag="ft")
        st = pst.tile([128, 4, D], _BF16, tag="st")
        for a in range(2):
            nc.sync.dma_start(
                ft[:, a * 2 : a * 2 + 2, :],
                feat[ds(sc * 512 + a * 256, 256), :].rearrange(
                    "(j b) d -> b j d", b=128
                ),
            )
            pd = pdiff.tile([128, 2, D], _F32, tag="pd")
            for jj in range(2):
                w = sc * 4 + a * 2 + jj
                nc.tensor.matmul(
                    pd[:, jj, :],
                    oht[:, w, :, :],
                    wtt[:, w, :, :],
                    start=True,
                    stop=True,
                    perf_mode=DR,
                )
            nc.vector.tensor_tensor(
                out=st[:, a * 2 : a * 2 + 2, :],
                in0=ft[:, a * 2 : a * 2 + 2, :],
                in1=pd[:, :, :],
                op=mybir.AluOpType.subtract,
            )
        dmp = pdump.tile([128, 4, D], _BF16, tag="dmp")
        nc.scalar.activation(
            dmp[:], st[:], AF.Square, accum_out=staging[:, sc : sc + 1]
        )

    def angular_half(h):
        pg = pgram.tile([CS, 500], _F32, tag="pg")
        for ki in range(4):
            nc.tensor.matmul(
                pg[:],
                cst[:, ki, :],
                ctn[:, ki, ds(500 * h, 500)],
                start=(ki == 0),
                stop=(ki == 3),
            )
        gs = pang.tile([CS, 500], _F32, tag="gs")
        nc.scalar.activation(
            gs[:],
            pg[:],
            AF.Square,
            bias=ctb[:CS, :],
            accum_out=staging[:CS, 16 + h : 17 + h],
        )

    for sc in range(NSC):
        superchunk(sc)
        if sc == 3:
            angular_half(0)
        elif sc == 8:
            angular_half(1)

    pf = pfin.tile([1, NSTG], _F32, tag="fin")
    nc.tensor.matmul(pf[:], ones[:], staging[:], start=True, stop=True)
    osb = const.tile([1, NSTG], _F32)
    nc.vector.tensor_copy(osb[:], pf[:])
    nc.sync.dma_start(out[:, :], osb[:, :])


def build():
    if "nc" in _NC_CACHE:
        return _NC_CACHE["nc"]
    nc = bacc.Bacc(
        "TRN2",
        target_bir_lowering=False,
        debug=False,
        enable_asserts=False,
        num_devices=N_CORES,
    )
    feat = nc.dram_tensor("feat", [BS, D], _F32, kind="ExternalInput").ap()
    oh = nc.dram_tensor("oh", [4, NW, 2, 128], _FP8, kind="ExternalInput").ap()
    wt = nc.dram_tensor("wt", [4, NW, 2, D], _FP8, kind="ExternalInput").ap()
    ctnT = nc.dram_tensor("ctnT", [128, 4, C], _BF16, kind="ExternalInput").ap()
    csT = nc.dram_tensor("csT", [128, 4, CS], _BF16, kind="ExternalInput").ap()
    out = nc.dram_tensor("out", [1, NSTG], _F32, kind="ExternalOutput").ap()
    with tile.TileContext(nc) as tc, ExitStack() as ctx:
        _build_body(ctx, tc, feat, oh, wt, ctnT, csT, out)
    nc.compile()
    _NC_CACHE["nc"] = nc
    return nc


def _dmajor(a):
    """[R, 512] -> [128, 4, R]: out[p, k, r] = a[r, 128*k + p]."""
    r = a.shape[0]
    return np.ascontiguousarray(a.T.reshape(4, 128, r).transpose(1, 0, 2))


def make_in_maps(y, feat, centers):
    feat = np.asarray(feat, dtype=np.float32)
    centers = np.asarray(centers, dtype=np.float32)
    y = np.asarray(y)

    order = np.argsort(y, kind="stable")
    ys = np.asarray(y)[order].astype(np.int32)

    norms = np.sqrt((centers.astype(np.float64) ** 2).sum(1)).astype(np.float32)
    ctn = (centers / norms[:, None]).astype(ml_dtypes.bfloat16)
    ctnT = _dmajor(ctn)  # [128, 4, 1000]
    c8 = centers.astype(ml_dtypes.float8_e4m3)

    in_maps = []
    for i in range(N_CORES):
        sl = slice(i * BS, (i + 1) * BS)
        fs = np.ascontiguousarray(feat[order[sl]])
        lid2 = ys[sl].reshape(NW, 128)
        c0 = lid2[:, 0]  # sorted -> min class of each window
        lid = lid2 - c0[:, None]
        if lid.max() >= WCAP:
            raise ValueError(f"window span {lid.max() + 1} exceeds WCAP={WCAP}")
        # local class l = p + 4*j lives at [p, j] (DoubleRow k-tile layout)
        l_of = np.arange(WCAP).reshape(2, 4).T  # [p, j] -> l
        rows = np.minimum(c0[None, :, None] + l_of[:, None, :], C - 1)  # [4,NW,2]
        wt = np.ascontiguousarray(c8[rows])  # [4, NW, 2, 512]
        oh = (l_of[:, None, :, None] == lid[None, :, None, :]).astype(
            ml_dtypes.float8_e4m3
        )  # [4, NW, 2, 128]
        csT = _dmajor(np.asarray(ctn[i * CS : (i + 1) * CS]))  # [128, 4, 125]
        in_maps.append(
            {"feat": fs, "oh": oh, "wt": wt, "ctnT": ctnT, "csT": csT}
        )
    return in_maps


def combine(outs):
    """outs: list of 8 [1,NSTG] f32 arrays -> scalar loss (np.float32)."""
    cen = 0.0
    ang = 0.0
    for o in outs:
        o = np.asarray(o, dtype=np.float64)
        cen += o[0, 0:16].sum()
        ang += o[0, 16:18].sum()
    ang -= C * (1.0 - CT) ** 2  # remove the diagonal (sim_ii == 1) terms
    loss = 0.5 * cen / B + ang / (0.5 * C * (C - 1))
    return np.float32(loss)


def kernel(y, feat, centers):
    nc = build()
    in_maps = make_in_maps(y, feat, centers)
    res = run_bass_kernel_spmd(nc, in_maps, core_ids=list(range(N_CORES)))
    return combine([res.results[i]["out"] for i in range(N_CORES)])


# revision 6
# speedup vs baseline: 1.6127x; 1.0105x over previous
"""Trainium2 Bass kernel for AngelLoss (center loss + angular loss).

loss = 0.5*sum((feat - centers[y])^2)/B
     + sum_offdiag((c_i.c_j/(|c_i||c_j|) - ct)^2) / (0.5*C*(C-1))

Strategy (8 NeuronCores):
  - The batch sum is order-invariant, so the host shards feat GLOBALLY
    SORTED by class (index prep only; all O(B*D) compute stays on
    device). Each 512-row superchunk then spans <= ~10 consecutive
    classes, and centers[y] is expanded on-device by a tiny one-hot
    matmul on TensorE against a 16-row class window -- no SWDGE gather.
    One-hot and window tables are fp8e4 with DoubleRow k-tiles.
  - feat streams in slab-contiguous tiles (ft[p,s,:] = row 4p+s) so each
    DMA is 128 x 8KB contiguous segments; the slot-s one-hot matmul
    gathers rows {4p+s} so PSUM partitions align with feat partitions.
  - Per slot pair: PSUM diff source = onehot^T @ window (TensorE),
    subtract vs feat (DVE), square+accumulate over 4 slots (ScalarE).
  - Angular term: center rows sharded (125 rows/core); the host uploads
    the normalized center table pre-transposed (d-major), so the
    125x1000 Gram slice is 8 direct matmuls -- no on-device transposes.
  - per-core [1,20] partial sums are combined on the host.
"""

from contextlib import ExitStack

import ml_dtypes
import numpy as np

import concourse.bass as bass
import concourse.tile as tile
from concourse import bacc, mybir
from concourse.bass import ds, ts
from concourse.bass_utils import run_bass_kernel_spmd

N_CORES = 8
B, C, D = 65536, 1000, 512
BS = B // N_CORES  # 8192 rows per core
CS = C // N_CORES  # 125 gram rows per core
NSC = 16  # superchunks of 512 rows
WCAP = 16  # max classes spanned by one 512-row sorted superchunk (seed-0: 10)
NSTG = 20  # staging columns: 0..15 center partials, 16..17 angular halves

# ct = 2*radius(C-1)^2 - 1 from the reference, evaluated in f64, cast f32.
CT = float(np.float32(-0.0010010010010047532))

_F32 = mybir.dt.float32
_BF16 = mybir.dt.bfloat16
_FP8 = mybir.dt.float8e4

_NC_CACHE = {}


def _build_body(ctx, tc, feat, oh, wt, ctnT, csT, out):
    nc = tc.nc
    AF = mybir.ActivationFunctionType
    DR = mybir.MatmulPerfMode.DoubleRow

    const = ctx.enter_context(tc.tile_pool(name="const", bufs=1))
    pfeat = ctx.enter_context(tc.tile_pool(name="feat", bufs=3))
    pst = ctx.enter_context(tc.tile_pool(name="st", bufs=3))
    pdump = ctx.enter_context(tc.tile_pool(name="dump", bufs=2))
    pang = ctx.enter_context(tc.tile_pool(name="ang", bufs=2))
    pdiff = ctx.enter_context(tc.tile_pool(name="diff", bufs=3, space="PSUM"))
    pgram = ctx.enter_context(tc.tile_pool(name="gram", bufs=1, space="PSUM"))
    pfin = ctx.enter_context(tc.tile_pool(name="fin", bufs=1, space="PSUM"))

    # small fp8 tables ride the fast sync queue ahead of the feat stream;
    # the 1MB angular table takes the (slower) scalar queue in parallel.
    oht = const.tile([8, NSC * 4, 2, 128], _FP8)
    nc.sync.dma_start(oht[:], oh[:, :, :, :])
    wtt = const.tile([8, NSC, 2, D], _FP8)
    nc.sync.dma_start(wtt[:], wt[:, :, :, :])
    ctn = const.tile([128, 4, C], _BF16)
    nc.scalar.dma_start(ctn[:], ctnT[:, :, :])
    cst = const.tile([128, 4, CS], _BF16)
    nc.scalar.dma_start(cst[:], csT[:, :, :])

    ones = const.tile([128, 1], _F32)
    nc.vector.memset(ones[:], 1.0)
    ctb = const.tile([128, 1], _F32)
    nc.vector.memset(ctb[:], -CT)
    staging = const.tile([128, NSTG], _F32)
    nc.vector.memset(staging[:], 0.0)

    def superchunk(sc):
        ft = pfeat.tile([128, 4, D], _F32, tag="ft")
        nc.sync.dma_start(
            ft[:], feat[ds(sc * 512, 512), :].rearrange("(p s) d -> p s d", p=128)
        )
        st = pst.tile([128, 4, D], _BF16, tag="st")
        for a in range(2):
            pd = pdiff.tile([128, 2, D], _F32, tag="pd")
            for jj in range(2):
                s = a * 2 + jj
                nc.tensor.matmul(
                    pd[:, jj, :],
                    oht[:, sc * 4 + s, :, :],
                    wtt[:, sc, :, :],
                    start=True,
                    stop=True,
                    perf_mode=DR,
                )
            nc.vector.tensor_tensor(
                out=st[:, a * 2 : a * 2 + 2, :],
                in0=ft[:, a * 2 : a * 2 + 2, :],
                in1=pd[:, :, :],
                op=mybir.AluOpType.subtract,
            )
        dmp = pdump.tile([128, 4, D], _BF16, tag="dmp")
        nc.scalar.activation(
            dmp[:], st[:], AF.Square, accum_out=staging[:, sc : sc + 1]
        )

    def angular_half(h):
        pg = pgram.tile([CS, 500], _F32, tag="pg")
        for ki in range(4):
            nc.tensor.matmul(
                pg[:],
                cst[:, ki, :],
                ctn[:, ki, ds(500 * h, 500)],
                start=(ki == 0),
                stop=(ki == 3),
            )
        gs = pang.tile([CS, 500], _F32, tag="gs")
        nc.scalar.activation(
            gs[:],
            pg[:],
            AF.Square,
            bias=ctb[:CS, :],
            accum_out=staging[:CS, 16 + h : 17 + h],
        )

    for sc in range(NSC):
        superchunk(sc)
        if sc == 7:
            angular_half(0)
        elif sc == 12:
            angular_half(1)

    pf = pfin.tile([1, NSTG], _F32, tag="fin")
    nc.tensor.matmul(pf[:], ones[:], staging[:], start=True, stop=True)
    osb = const.tile([1, NSTG], _F32)
    nc.vector.tensor_copy(osb[:], pf[:])
    nc.sync.dma_start(out[:, :], osb[:, :])


def build():
    if "nc" in _NC_CACHE:
        return _NC_CACHE["nc"]
    nc = bacc.Bacc(
        "TRN2",
        target_bir_lowering=False,
        debug=False,
        enable_asserts=False,
        num_devices=N_CORES,
    )
    feat = nc.dram_tensor("feat", [BS, D], _F32, kind="ExternalInput").ap()
    oh = nc.dram_tensor("oh", [8, NSC * 4, 2, 128], _FP8, kind="ExternalInput").ap()
    wt = nc.dram_tensor("wt", [8, NSC, 2, D], _FP8, kind="ExternalInput").ap()
    ctnT = nc.dram_tensor("ctnT", [128, 4, C], _BF16, kind="ExternalInput").ap()
    csT = nc.dram_tensor("csT", [128, 4, CS], _BF16, kind="ExternalInput").ap()
    out = nc.dram_tensor("out", [1, NSTG], _F32, kind="ExternalOutput").ap()
    with tile.TileContext(nc) as tc, ExitStack() as ctx:
        _build_body(ctx, tc, feat, oh, wt, ctnT, csT, out)
    nc.compile()
    _NC_CACHE["nc"] = nc
    return nc


def _dmajor(a):
    """[R, 512] -> [128, 4, R]: out[p, k, r] = a[r, 128*k + p]."""
    r = a.shape[0]
    return np.ascontiguousarray(a.T.reshape(4, 128, r).transpose(1, 0, 2))


def make_in_maps(y, feat, centers):
    feat = np.asarray(feat, dtype=np.float32)
    centers = np.asarray(centers, dtype=np.float32)
    y = np.asarray(y)

    order = np.argsort(y, kind="stable")
    ys = np.asarray(y)[order].astype(np.int32)

    norms = np.sqrt((centers.astype(np.float64) ** 2).sum(1)).astype(np.float32)
    ctn = (centers / norms[:, None]).astype(ml_dtypes.bfloat16)
    ctnT = _dmajor(ctn)  # [128, 4, 1000]
    c8 = centers.astype(ml_dtypes.float8_e4m3)
    # local class l = kp + 8*kj lives at [kp, kj] (DoubleRow k-tile layout)
    l_of = np.arange(WCAP).reshape(2, 8).T  # [kp, kj] -> l

    in_maps = []
    for i in range(N_CORES):
        sl = slice(i * BS, (i + 1) * BS)
        fs = np.ascontiguousarray(feat[order[sl]])
        ysc = ys[sl]
        c0 = ysc.reshape(NSC, 512)[:, 0]  # sorted -> min class per superchunk
        lid = ysc.reshape(NSC, 128, 4) - c0[:, None, None]  # [sc, p, s]
        if lid.max() >= WCAP:
            raise ValueError(f"window span {lid.max() + 1} exceeds WCAP={WCAP}")
        lid_w = lid.transpose(0, 2, 1).reshape(NSC * 4, 128)  # [w=(sc,s), p]
        oh = (l_of[:, None, :, None] == lid_w[None, :, None, :]).astype(
            ml_dtypes.float8_e4m3
        )  # [8, NSC*4, 2, 128]
        rows = np.minimum(c0[None, :, None] + l_of[:, None, :], C - 1)  # [8,NSC,2]
        wt = np.ascontiguousarray(c8[rows])  # [8, NSC, 2, 512]
        csT = _dmajor(np.asarray(ctn[i * CS : (i + 1) * CS]))  # [128, 4, 125]
        in_maps.append(
            {"feat": fs, "oh": oh, "wt": wt, "ctnT": ctnT, "csT": csT}
        )
    return in_maps


def combine(outs):
    """outs: list of 8 [1,NSTG] f32 arrays -> scalar loss (np.float32)."""
    cen = 0.0
    ang = 0.0
    for o in outs:
        o = np.asarray(o, dtype=np.float64)
        cen += o[0, 0:16].sum()
        ang += o[0, 16:18].sum()
    ang -= C * (1.0 - CT) ** 2  # remove the diagonal (sim_ii == 1) terms
    loss = 0.5 * cen / B + ang / (0.5 * C * (C - 1))
    return np.float32(loss)


def kernel(y, feat, centers):
    nc = build()
    in_maps = make_in_maps(y, feat, centers)
    res = run_bass_kernel_spmd(nc, in_maps, core_ids=list(range(N_CORES)))
    return combine([res.results[i]["out"] for i in range(N_CORES)])
